# revision 33
# baseline (speedup 1.0000x reference)
"""Trainium2 Bass kernel for nn_MultiHeadAttention (B=4, S=2048, D=768, H=12).

Sharding: 8 cores = 4 batches x 2 head-groups (6 heads each).
Per core (batch b, group g):
  QT/KT = Wg @ x_b^T   [384, 2048]  - fp8e4 DoubleRow matmuls (weights*8,
          256-deep contraction per instr), bias added on DVE, bf16 out
  V     = x_b @ Wv_g^T - bf16, 16 tiles [128, 6*65], 65th col = 1.0 (denom);
          V bias folded into bo_eff = Wo_g @ bv_g + bo/2 host-side
  attention, software-pipelined over (qc in 4, head-pair hp in 3):
    per chunk slot: two row-tiled co-issued K=64 score matmuls (head A rows
    0-63 -> grp0, head B rows 64-127 -> grp64) into sAB [128, 1024] PSUM
    (bufs=2); one ACT exp instr [128, 1024] -> E bf16; previous head-pair's
    PV matmuls (M=65, ones-column -> denominator at row 64) interleave in the
    same slots, as do dripped projection / o-proj blocks (keeps the exp
    stream gapless and the PE queue saturated)
    normalize: denom row -> DVE copy + reciprocal_approx_fast -> gpsimd
    partition_broadcast -> DVE mult -> attn bf16
  o-proj per qc: outT[768, 512] bf16 chunks, dripped into later iterations
Host sums the two bf16 partial outT per batch (f32) and transposes back.

Measured: ~256-262 us HW exec (baseline 340 us), rel err 1.61e-2 (gate 2e-2).
Set QK_FP8 = False for an all-bf16 variant (~270 us, rel err 4.5e-3).
"""

import sys

import numpy as np
import ml_dtypes

if "/opt/trn_rl_repo" not in sys.path:
    sys.path.insert(0, "/opt/trn_rl_repo")

import concourse.bass as bass
import concourse.bacc as bacc
import concourse.mybir as mybir
import concourse.tile as tile
from concourse.bass_utils import run_bass_kernel_spmd

B, S, DM, NH, DK = 4, 2048, 768, 12, 64
NCORES = 8
HLOC = 6            # heads per core
GD = HLOC * DK      # 384
P = 128
NXT = DM // P       # 6 contraction tiles over d_model
NPT = GD // P       # 3 head-pair tiles (2 heads each)
NKT = S // P        # 16 k chunks
QC = 512            # q chunk
NQC = S // QC       # 4
VW = 65             # V cols per head (64 + ones)
VROW = HLOC * VW    # 390

F32 = mybir.dt.float32
BF16 = mybir.dt.bfloat16
FP8 = mybir.dt.float8e4
DR = mybir.MatmulPerfMode.DoubleRow
QK_FP8 = True
PV_COLTILE = True
NPFP8 = ml_dtypes.float8_e4m3
EXP = mybir.ActivationFunctionType.Exp
NPBF16 = ml_dtypes.bfloat16

_NC_CACHE = {}


def build_nc():
    nc = bacc.Bacc()

    xT = nc.declare_dram_parameter("xT", [P, NXT * S], BF16, isOutput=False)
    if QK_FP8:
        x8T = nc.declare_dram_parameter("x8T", [P, 6 * S], FP8, isOutput=False)
        wq8T = nc.declare_dram_parameter("wq8T", [P, 6 * GD], FP8, isOutput=False)
        wk8T = nc.declare_dram_parameter("wk8T", [P, 6 * GD], FP8, isOutput=False)
    wqT = nc.declare_dram_parameter("wqT", [P, NXT * GD], BF16, isOutput=False)
    wkT = nc.declare_dram_parameter("wkT", [P, NXT * GD], BF16, isOutput=False)
    wvT = nc.declare_dram_parameter("wvT", [P, NXT * GD], BF16, isOutput=False)
    woT = nc.declare_dram_parameter("woT", [P, NPT * DM], BF16, isOutput=False)
    pb = nc.declare_dram_parameter("pb", [P, 12], F32, isOutput=False)
    outT = nc.declare_dram_parameter("outT", [DM, S], BF16, isOutput=True)

    with tile.TileContext(nc) as tc:
        with (
            nc.allow_low_precision(reason="bf16 pipeline is intended"),
            tc.tile_pool(name="persist", bufs=1) as pp,
            tc.tile_pool(name="xpool", bufs=1) as xp,
            tc.tile_pool(name="epool", bufs=1) as ep,
            tc.tile_pool(name="work", bufs=1) as wp,
            tc.tile_pool(name="psum", bufs=1, space=bass.MemorySpace.PSUM) as psp,
        ):
            # ---- DMA loads (one per tensor; sync dispatch is serialized) ----
            pb_t = pp.tile([P, 12], F32, tag="pb", name="pb_t")
            nc.sync.dma_start(pb_t[:], pb[:])
            if QK_FP8:
                wk8_all = pp.tile([P, 3, 2, GD], FP8, tag="wk8", name="wk8_all")
                nc.sync.dma_start(wk8_all[:], wk8T[:])
                x8_all = xp.tile([P, 3, 2, S], FP8, tag="x8", name="x8_all")
                # qc0 columns of all 6 blocks first: unblocks the prefix
                # K/Q projections after 384 KB instead of 1.5 MB
                x8d = x8T.rearrange("p (b s) -> p b s", b=6)
                nc.sync.dma_start(
                    x8_all[:, :, :, 0:QC].rearrange("p i j s -> p (i j) s"),
                    x8d[:, :, 0:QC],
                )
                for i in range(3):
                    nc.sync.dma_start(
                        x8_all[:, i, :, QC:S],
                        x8d[:, i * 2 : (i + 1) * 2, QC:S],
                    )
                wq8_all = pp.tile([P, 3, 2, GD], FP8, tag="wq8", name="wq8_all")
                nc.sync.dma_start(wq8_all[:], wq8T[:])
            wk_all = pp.tile([P, NXT, GD], BF16, tag="wk", name="wk_all")
            if not QK_FP8:
                nc.sync.dma_start(wk_all[:], wkT[:])
            wq_all = pp.tile([P, NXT, GD], BF16, tag="wq", name="wq_all")
            if not QK_FP8:
                nc.sync.dma_start(wq_all[:], wqT[:])
            wv_all = pp.tile([P, NXT, GD], BF16, tag="wv", name="wv_all")
            nc.sync.dma_start(wv_all[:], wvT[:])
            xt_all = xp.tile([P, NXT, S], BF16, tag="xt", name="xt_all")
            for i in range(NXT):
                nc.sync.dma_start(xt_all[:, i, :], xT[:, i * S : (i + 1) * S])
            wo_all = pp.tile([P, NPT, DM], BF16, tag="wo", name="wo_all")
            nc.sync.dma_start(wo_all[:], woT[:])
            wk_t = [wk_all[:, i, :] for i in range(NXT)]
            wq_t = [wq_all[:, i, :] for i in range(NXT)]
            wv_t = [wv_all[:, i, :] for i in range(NXT)]
            xt = [xt_all[:, i, :] for i in range(NXT)]
            wo_t = [wo_all[:, j, :] for j in range(NPT)]

            # ---- persistent SBUF tensors ----
            ones_t = pp.tile([P, 1], BF16, tag="ones", name="ones_t")
            nc.vector.memset(ones_t[:], 1.0)
            # PE warm-up during DMA wait: keeps HAM at full clock for the
            # first real matmuls (no data deps - memset scratch input)
            wrm = wp.tile([P, QC], BF16, tag="wrm", bufs=1, name="wrm")
            nc.vector.memset(wrm[:], 0.5)
            # preload the exp activation table during the DMA wait
            wex = wp.tile([1, 1], BF16, tag="wex", bufs=1, name="wex")
            nc.scalar.activation(wex[:], wrm[0:1, 0:1], EXP, scale=1.0)
            wps = psp.tile([1, QC], F32, tag="acc", bufs=4, name="warm_ps")
            for i in range(24):
                nc.tensor.matmul(wps[:], ones_t[:], wrm[:], start=True, stop=True,
                                 skip_group_check=True)
            KT = [pp.tile([P, S], BF16, tag=f"KT{j}", name=f"KT{j}") for j in range(NPT)]
            QT = [pp.tile([P, S], BF16, tag=f"QT{j}", name=f"QT{j}") for j in range(NPT)]
            V = [pp.tile([P, VROW], BF16, tag=f"V{c}", name=f"V{c}") for c in range(NKT)]
            attn = [pp.tile([P, S], BF16, tag=f"attn{j}", name=f"attn{j}") for j in range(NPT)]

            def proj(dst_tiles, w_tiles, bias_base, hp, qc):
                """dst[hp][:, qc] = w^T @ x + bias  (one [128,512] psum)."""
                ps = psp.tile([P, QC], F32, tag="acc", bufs=4, name=f"pj{bias_base}{hp}_{qc}")
                qsl = slice(qc * QC, (qc + 1) * QC)
                if QK_FP8 and w_tiles in (wq8_all, wk8_all):
                    for i in range(3):
                        nc.tensor.matmul(
                            ps[:],
                            w_tiles[:, i, :, hp * P : (hp + 1) * P],
                            x8_all[:, i, :, qsl],
                            start=(i == 0),
                            stop=(i == 2),
                            perf_mode=DR,
                        )
                else:
                    for kt in range(NXT):
                        nc.tensor.matmul(
                            ps[:],
                            w_tiles[kt][:, hp * P : (hp + 1) * P],
                            xt[kt][:, qsl],
                            start=(kt == 0),
                            stop=(kt == NXT - 1),
                        )
                nc.vector.tensor_scalar_add(
                    dst_tiles[hp][:, qsl], ps[:], pb_t[:, bias_base + hp : bias_base + hp + 1]
                )

            def vproj(c):
                """V[c] = (x_chunk @ Wv^T | ones) as [128, 6*65] bf16."""
                ps = psp.tile([P, QC], F32, tag="acc", bufs=4, name=f"vp{c}")
                for kt in range(NXT):
                    nc.tensor.matmul(
                        ps[:, 0:GD],
                        xt[kt][:, c * P : (c + 1) * P],
                        wv_t[kt][:],
                        start=(kt == 0),
                        stop=(kt == NXT - 1),
                    )
                vv = V[c].rearrange("p (h c) -> p h c", h=HLOC)
                nc.vector.tensor_copy(
                    vv[:, :, 0:DK], ps[:, 0:GD].rearrange("p (h c) -> p h c", h=HLOC)
                )
                nc.vector.memset(vv[:, :, DK : DK + 1], 1.0)

            def oproj(qc, mts):
                qsl = slice(qc * QC, (qc + 1) * QC)
                for mt in mts:
                    po = psp.tile([P, QC], F32, tag="acc", bufs=4, name=f"po{mt}_{qc}")
                    for j in range(NPT):
                        nc.tensor.matmul(
                            po[:],
                            wo_t[j][:, mt * P : (mt + 1) * P],
                            attn[j][:, qsl],
                            start=(j == 0),
                            stop=(j == NPT - 1),
                        )
                    osb = wp.tile([P, QC], BF16, tag="os", bufs=4, name=f"os{mt}_{qc}")
                    nc.vector.tensor_scalar_add(osb[:], po[:], pb_t[:, 6 + mt : 7 + mt])
                    nc.sync.dma_start(outT[mt * P : (mt + 1) * P, qsl], osb[:])

            # ---- emission helpers ----
            osb_all = [None] * NQC

            def oproj_mt(qc, mt):
                qsl = slice(qc * QC, (qc + 1) * QC)
                if osb_all[qc] is None:
                    osb_all[qc] = wp.tile([P, NXT, QC], BF16, tag="os", bufs=2, name=f"os{qc}")
                po = psp.tile([P, QC], F32, tag="acc", bufs=4, name=f"po{mt}_{qc}")
                for j in range(NPT):
                    nc.tensor.matmul(
                        po[:],
                        wo_t[j][:, mt * P : (mt + 1) * P],
                        attn[j][:, qsl],
                        start=(j == 0),
                        stop=(j == NPT - 1),
                    )
                nc.vector.tensor_scalar_add(
                    osb_all[qc][:, mt, :], po[:], pb_t[:, 6 + mt : 7 + mt]
                )
                if mt == NXT - 1:
                    nc.sync.dma_start(
                        outT.rearrange("(m p) s -> p m s", p=P)[:, :, qsl],
                        osb_all[qc][:],
                    )

            def emit_pv(pv, c):
                E, atA, atB, hA, hB = pv["E"], pv["atA"], pv["atB"], pv["hA"], pv["hB"]
                nc.tensor.matmul(
                    atA[:], V[c][:, hA * VW : (hA + 1) * VW], E[c][:, 0:QC],
                    start=(c == 0), stop=(c == NKT - 1), skip_group_check=True,
                )
                nc.tensor.matmul(
                    atB[:], V[c][:, hB * VW : (hB + 1) * VW], E[c][:, QC : 2 * QC],
                    start=(c == 0), stop=(c == NKT - 1), skip_group_check=True,
                )

            def emit_normalize(pv):
                atA, atB = pv["atA"], pv["atB"]
                _qc, _hp = pv["qc"], pv["hp"]
                qsl = slice(_qc * QC, (_qc + 1) * QC)
                heads = []
                for at, rows in ((atA, slice(0, DK)), (atB, slice(DK, P))):
                    dsb = wp.tile([1, QC], F32, tag="dsb", bufs=4, name=f"d{_qc}_{_hp}_{rows.start}")
                    nc.vector.tensor_copy(dsb[:], at[DK:VW, :])
                    heads.append((at, rows, dsb))
                rs = []
                for at, rows, dsb in heads:
                    r = wp.tile([1, QC], F32, tag="r", bufs=4, name=f"r{_qc}_{_hp}_{rows.start}")
                    nc.vector.reciprocal_approx_fast(r[:], dsb[:])
                    rs.append(r)
                rbs = []
                for (at, rows, dsb), r in zip(heads, rs):
                    rb = wp.tile([DK, QC], F32, tag="rb", bufs=4, name=f"rb{_qc}_{_hp}_{rows.start}")
                    nc.gpsimd.partition_broadcast(rb[:], r[:], channels=DK)
                    rbs.append(rb)
                for (at, rows, dsb), rb in zip(heads, rbs):
                    nc.vector.tensor_mul(attn[_hp][rows, qsl], at[0:DK, :], rb[:])

            # ---- prefix: only what the first score chunks need ----
            # fused K(0,0)/Q(0,0): interleave MMs so both consume each x8
            # piece as it arrives instead of Q re-waiting the DMA chain
            if QK_FP8:
                psk = psp.tile([P, QC], F32, tag="acc", bufs=4, name="pfx_k")
                psq = psp.tile([P, QC], F32, tag="acc", bufs=4, name="pfx_q")
                for i in range(3):
                    nc.tensor.matmul(
                        psk[:], wk8_all[:, i, :, 0:P], x8_all[:, i, :, 0:QC],
                        start=(i == 0), stop=(i == 2), perf_mode=DR,
                    )
                    nc.tensor.matmul(
                        psq[:], wq8_all[:, i, :, 0:P], x8_all[:, i, :, 0:QC],
                        start=(i == 0), stop=(i == 2), perf_mode=DR,
                    )
                nc.vector.tensor_scalar_add(KT[0][:, 0:QC], psk[:], pb_t[:, 3:4])
                nc.vector.tensor_scalar_add(QT[0][:, 0:QC], psq[:], pb_t[:, 0:1])
            else:
                proj(KT, wk_t, 3, 0, 0)
                proj(QT, wq_t, 0, 0, 0)

            # ---- drip schedule: deferred PE work, one block per chunk slot ----
            drip = [[] for _ in range(12)]
            drip[0] = (
                [lambda q2=q2: proj(KT, wk8_all if QK_FP8 else wk_t, 3, 0, q2) for q2 in range(1, NQC)]
                + [lambda q2=q2: proj(KT, wk8_all if QK_FP8 else wk_t, 3, 1, q2) for q2 in range(NQC)]
                + [lambda: proj(QT, wq8_all if QK_FP8 else wq_t, 0, 1, 0)]
                + [lambda c=c: vproj(c) for c in range(8)]
            )
            drip[1] = (
                [lambda c=c: vproj(c) for c in range(8, 12)]
                + [lambda: proj(KT, wk8_all if QK_FP8 else wk_t, 3, 2, 0), lambda: proj(QT, wq8_all if QK_FP8 else wq_t, 0, 2, 0)]
                + [lambda c=c: vproj(c) for c in range(12, NKT)]
            )
            drip[2] = (
                [lambda q2=q2: proj(KT, wk8_all if QK_FP8 else wk_t, 3, 2, q2) for q2 in range(1, NQC)]
                + [lambda: proj(QT, wq8_all if QK_FP8 else wq_t, 0, 0, 1)]
            )
            drip[3] = [
                lambda: proj(QT, wq8_all if QK_FP8 else wq_t, 0, 1, 1),
                lambda: proj(QT, wq8_all if QK_FP8 else wq_t, 0, 2, 1),
                lambda: proj(QT, wq8_all if QK_FP8 else wq_t, 0, 0, 2),
            ]
            drip[4] = [lambda mt=mt: oproj_mt(0, mt) for mt in range(3)] + [
                lambda: proj(QT, wq8_all if QK_FP8 else wq_t, 0, 1, 2)
            ]
            drip[5] = [lambda mt=mt: oproj_mt(0, mt) for mt in range(3, NXT)] + [
                lambda: proj(QT, wq8_all if QK_FP8 else wq_t, 0, 2, 2), lambda: proj(QT, wq8_all if QK_FP8 else wq_t, 0, 0, 3)
            ]
            drip[6] = [lambda: proj(QT, wq8_all if QK_FP8 else wq_t, 0, 1, 3)]
            drip[7] = [lambda mt=mt: oproj_mt(1, mt) for mt in range(3)] + [
                lambda: proj(QT, wq8_all if QK_FP8 else wq_t, 0, 2, 3)
            ]
            drip[8] = [lambda mt=mt: oproj_mt(1, mt) for mt in range(3, NXT)]
            drip[10] = [lambda mt=mt: oproj_mt(2, mt) for mt in range(NXT)]

            # ---- attention: slot-pipelined emission ----
            prev = None
            iters = [(qc, hp) for qc in range(NQC) for hp in range(NPT)]
            for it_idx, (qc, hp) in enumerate(iters):
                qsl = slice(qc * QC, (qc + 1) * QC)
                hA, hB = 2 * hp, 2 * hp + 1
                cur = {
                    "E": [],
                    "atA": psp.tile([VW, QC], F32, tag="acc", bufs=4, name=f"atA{qc}_{hp}"),
                    "atB": psp.tile([VW, QC], F32, tag="acc", bufs=4, name=f"atB{qc}_{hp}"),
                    "hA": hA, "hB": hB, "qc": qc, "hp": hp,
                }
                dq = list(drip[it_idx])
                for c in range(NKT):
                    sAB = psp.tile([P, 2 * QC], F32, tag="sAB", bufs=2, name=f"s{qc}_{hp}_{c}")
                    ksl = slice(c * P, (c + 1) * P)
                    nc.tensor.matmul(sAB[:, 0:QC], KT[hp][0:DK, ksl], QT[hp][0:DK, qsl])
                    nc.tensor.matmul(
                        sAB[:, QC : 2 * QC], KT[hp][DK:P, ksl], QT[hp][DK:P, qsl]
                    )
                    e = ep.tile([P, 2 * QC], BF16, tag="E", bufs=20, name=f"e{qc}_{hp}_{c}")
                    nc.scalar.activation(e[:], sAB[:], EXP, scale=(1.0 / (DK * DK * 1.0)) if QK_FP8 else (1.0 / DK))
                    cur["E"].append(e)
                    if prev is not None:
                        emit_pv(prev, c)
                    if it_idx == len(iters) - 1 and c >= 1:
                        emit_pv(cur, c - 1)
                    if dq:
                        dq.pop(0)()
                while dq:
                    dq.pop(0)()
                if prev is not None:
                    emit_normalize(prev)
                prev = cur
            # flush last head-pair (chunks 0..14 already emitted in-loop)
            emit_pv(prev, NKT - 1)
            emit_normalize(prev)
            # keep PE warm through the normalize pause so oproj(3) runs at
            # full clock
            for i in range(10):
                nc.tensor.matmul(wps[:], ones_t[:], wrm[:], start=True, stop=True,
                                 skip_group_check=True)
            for mt in range(NXT):
                oproj_mt(3, mt)

    nc.compile()
    return nc


def make_in_maps(x, Wq, bq, Wk, bk, Wv, bv, Wo, bo):
    in_maps = []
    for c in range(NCORES):
        b, g = c // 2, c % 2
        sl = slice(g * GD, (g + 1) * GD)
        pbv = np.zeros((P, 12), np.float32)
        bo_eff = Wo[:, sl].astype(np.float64) @ bv[sl].astype(np.float64) + bo / 2.0
        qs = 8.0 if QK_FP8 else 1.0
        for j in range(NPT):
            pbv[:, 0 + j] = bq[sl][j * P : (j + 1) * P] * qs
            pbv[:, 3 + j] = bk[sl][j * P : (j + 1) * P] * qs
        for j in range(NXT):
            pbv[:, 6 + j] = bo_eff[j * P : (j + 1) * P]
        def blk(a, rows):
            # [rows*P, C] -> [P, rows*C] with block i at cols [i*C:(i+1)*C]
            r, cdim = a.shape
            return np.ascontiguousarray(
                a.reshape(rows, P, cdim).transpose(1, 0, 2).reshape(P, rows * cdim)
            )

        m = {}
        if QK_FP8:
            def dr_blk(a):
                # [768, C] -> [P, 3, 2, C] with d = i*256 + t*128 + p
                cdim = a.shape[1]
                return np.ascontiguousarray(
                    a.reshape(3, 2, P, cdim).transpose(2, 0, 1, 3).reshape(P, 3 * 2 * cdim)
                )
            m["x8T"] = dr_blk(x[b].T).astype(NPFP8)
            m["wq8T"] = dr_blk(Wq[sl, :].T * 8.0).astype(NPFP8)
            m["wk8T"] = dr_blk(Wk[sl, :].T * 8.0).astype(NPFP8)
        in_maps.append(
            {
                **m,
                "xT": blk(x[b].T, NXT).astype(NPBF16),
                "wqT": blk(Wq[sl, :].T, NXT).astype(NPBF16),
                "wkT": blk(Wk[sl, :].T, NXT).astype(NPBF16),
                "wvT": blk(Wv[sl, :].T, NXT).astype(NPBF16),
                "woT": blk(Wo[:, sl].T, NPT).astype(NPBF16),
                "pb": pbv,
            }
        )
    return in_maps


def kernel(x, Wq, bq, Wk, bk, Wv, bv, Wo, bo, _trace=False):
    x = np.asarray(x, np.float32)
    args = [np.asarray(a, np.float32) for a in (Wq, bq, Wk, bk, Wv, bv, Wo, bo)]
    if "nc" not in _NC_CACHE:
        _NC_CACHE["nc"] = build_nc()
    nc = _NC_CACHE["nc"]
    in_maps = make_in_maps(x, *args)
    res = run_bass_kernel_spmd(nc, in_maps, core_ids=list(range(NCORES)), trace=_trace)
    _NC_CACHE["last_result"] = res
    out = np.empty((B, S, DM), np.float32)
    for b in range(B):
        out[b] = (
            res.results[2 * b]["outT"].astype(np.float32)
            + res.results[2 * b + 1]["outT"].astype(np.float32)
        ).T
    return out


# revision 34
# speedup vs baseline: 1.0077x; 1.0077x over previous
"""Trainium2 Bass kernel for nn_MultiHeadAttention (B=4, S=2048, D=768, H=12).

Sharding: 8 cores = 4 batches x 2 head-groups (6 heads each).
Per core (batch b, group g):
  QT/KT = Wg @ x_b^T   [384, 2048]  - fp8e4 DoubleRow matmuls (weights*8,
          256-deep contraction per instr), bias added on DVE, bf16 out
  V     = x_b @ Wv_g^T - bf16, 16 tiles [128, 6*65], 65th col = 1.0 (denom);
          V bias folded into bo_eff = Wo_g @ bv_g + bo/2 host-side
  attention, software-pipelined over (qc in 4, head-pair hp in 3):
    per chunk slot: two row-tiled co-issued K=64 score matmuls (head A rows
    0-63 -> grp0, head B rows 64-127 -> grp64) into sAB [128, 1024] PSUM
    (bufs=2); one ACT exp instr [128, 1024] -> E bf16; previous head-pair's
    PV matmuls (M=65, ones-column -> denominator at row 64) interleave in the
    same slots, as do dripped projection / o-proj blocks (keeps the exp
    stream gapless and the PE queue saturated)
    normalize: denom row -> DVE copy + reciprocal_approx_fast -> gpsimd
    partition_broadcast -> DVE mult -> attn bf16
  o-proj per qc: outT[768, 512] bf16 chunks, dripped into later iterations
Host sums the two bf16 partial outT per batch (f32) and transposes back.

Measured: ~256-262 us HW exec (baseline 340 us), rel err 1.61e-2 (gate 2e-2).
Set QK_FP8 = False for an all-bf16 variant (~270 us, rel err 4.5e-3).
"""

import sys

import numpy as np
import ml_dtypes

if "/opt/trn_rl_repo" not in sys.path:
    sys.path.insert(0, "/opt/trn_rl_repo")

import concourse.bass as bass
import concourse.bacc as bacc
import concourse.mybir as mybir
import concourse.tile as tile
from concourse.bass_utils import run_bass_kernel_spmd

B, S, DM, NH, DK = 4, 2048, 768, 12, 64
NCORES = 8
HLOC = 6            # heads per core
GD = HLOC * DK      # 384
P = 128
NXT = DM // P       # 6 contraction tiles over d_model
NPT = GD // P       # 3 head-pair tiles (2 heads each)
NKT = S // P        # 16 k chunks
QC = 512            # q chunk
NQC = S // QC       # 4
VW = 65             # V cols per head (64 + ones)
VROW = HLOC * VW    # 390

F32 = mybir.dt.float32
BF16 = mybir.dt.bfloat16
FP8 = mybir.dt.float8e4
DR = mybir.MatmulPerfMode.DoubleRow
QK_FP8 = True
PV_COLTILE = True
NPFP8 = ml_dtypes.float8_e4m3
EXP = mybir.ActivationFunctionType.Exp
NPBF16 = ml_dtypes.bfloat16

_NC_CACHE = {}


def build_nc():
    nc = bacc.Bacc()

    xT = nc.declare_dram_parameter("xT", [P, NXT * S], BF16, isOutput=False)
    if QK_FP8:
        x8T = nc.declare_dram_parameter("x8T", [P, 6 * S], FP8, isOutput=False)
        wq8T = nc.declare_dram_parameter("wq8T", [P, 6 * GD], FP8, isOutput=False)
        wk8T = nc.declare_dram_parameter("wk8T", [P, 6 * GD], FP8, isOutput=False)
    wqT = nc.declare_dram_parameter("wqT", [P, NXT * GD], BF16, isOutput=False)
    wkT = nc.declare_dram_parameter("wkT", [P, NXT * GD], BF16, isOutput=False)
    wvT = nc.declare_dram_parameter("wvT", [P, NXT * GD], BF16, isOutput=False)
    woT = nc.declare_dram_parameter("woT", [P, NPT * DM], BF16, isOutput=False)
    pb = nc.declare_dram_parameter("pb", [P, 12], F32, isOutput=False)
    outT = nc.declare_dram_parameter("outT", [DM, S], BF16, isOutput=True)

    with tile.TileContext(nc) as tc:
        with (
            nc.allow_low_precision(reason="bf16 pipeline is intended"),
            tc.tile_pool(name="persist", bufs=1) as pp,
            tc.tile_pool(name="xpool", bufs=1) as xp,
            tc.tile_pool(name="epool", bufs=1) as ep,
            tc.tile_pool(name="work", bufs=1) as wp,
            tc.tile_pool(name="psum", bufs=1, space=bass.MemorySpace.PSUM) as psp,
        ):
            # ---- DMA loads (one per tensor; sync dispatch is serialized) ----
            pb_t = pp.tile([P, 12], F32, tag="pb", name="pb_t")
            nc.sync.dma_start(pb_t[:], pb[:])
            if QK_FP8:
                wk8_all = pp.tile([P, 3, 2, GD], FP8, tag="wk8", name="wk8_all")
                nc.sync.dma_start(wk8_all[:], wk8T[:])
                x8_all = xp.tile([P, 3, 2, S], FP8, tag="x8", name="x8_all")
                # qc0 columns of all 6 blocks first: unblocks the prefix
                # K/Q projections after 384 KB instead of 1.5 MB
                x8d = x8T.rearrange("p (b s) -> p b s", b=6)
                nc.sync.dma_start(
                    x8_all[:, :, :, 0:QC].rearrange("p i j s -> p (i j) s"),
                    x8d[:, :, 0:QC],
                )
                for i in range(3):
                    nc.sync.dma_start(
                        x8_all[:, i, :, QC:S],
                        x8d[:, i * 2 : (i + 1) * 2, QC:S],
                    )
                wq8_all = pp.tile([P, 3, 2, GD], FP8, tag="wq8", name="wq8_all")
                nc.sync.dma_start(wq8_all[:], wq8T[:])
            wk_all = pp.tile([P, NXT, GD], BF16, tag="wk", name="wk_all")
            if not QK_FP8:
                nc.sync.dma_start(wk_all[:], wkT[:])
            wq_all = pp.tile([P, NXT, GD], BF16, tag="wq", name="wq_all")
            if not QK_FP8:
                nc.sync.dma_start(wq_all[:], wqT[:])
            wv_all = pp.tile([P, NXT, GD], BF16, tag="wv", name="wv_all")
            nc.sync.dma_start(wv_all[:], wvT[:])
            xt_all = xp.tile([P, NXT, S], BF16, tag="xt", name="xt_all")
            for i in range(NXT):
                nc.sync.dma_start(xt_all[:, i, :], xT[:, i * S : (i + 1) * S])
            wo_all = pp.tile([P, NPT, DM], BF16, tag="wo", name="wo_all")
            nc.sync.dma_start(wo_all[:], woT[:])
            wk_t = [wk_all[:, i, :] for i in range(NXT)]
            wq_t = [wq_all[:, i, :] for i in range(NXT)]
            wv_t = [wv_all[:, i, :] for i in range(NXT)]
            xt = [xt_all[:, i, :] for i in range(NXT)]
            wo_t = [wo_all[:, j, :] for j in range(NPT)]

            # ---- persistent SBUF tensors ----
            ones_t = pp.tile([P, 1], BF16, tag="ones", name="ones_t")
            nc.vector.memset(ones_t[:], 1.0)
            # PE warm-up during DMA wait: keeps HAM at full clock for the
            # first real matmuls (no data deps - memset scratch input)
            wrm = wp.tile([P, QC], BF16, tag="wrm", bufs=1, name="wrm")
            nc.vector.memset(wrm[:], 0.5)
            # preload the exp activation table during the DMA wait
            wex = wp.tile([1, 1], BF16, tag="wex", bufs=1, name="wex")
            nc.scalar.activation(wex[:], wrm[0:1, 0:1], EXP, scale=1.0)
            wps = psp.tile([1, QC], F32, tag="acc", bufs=4, name="warm_ps")
            for i in range(24):
                nc.tensor.matmul(wps[:], ones_t[:], wrm[:], start=True, stop=True,
                                 skip_group_check=True)
            KT = [pp.tile([P, S], BF16, tag=f"KT{j}", name=f"KT{j}") for j in range(NPT)]
            QT = [pp.tile([P, S], BF16, tag=f"QT{j}", name=f"QT{j}") for j in range(NPT)]
            V = [pp.tile([P, VROW], BF16, tag=f"V{c}", name=f"V{c}") for c in range(NKT)]
            attn = [pp.tile([P, S], BF16, tag=f"attn{j}", name=f"attn{j}") for j in range(NPT)]

            def proj(dst_tiles, w_tiles, bias_base, hp, qc):
                """dst[hp][:, qc] = w^T @ x + bias  (one [128,512] psum)."""
                ps = psp.tile([P, QC], F32, tag="acc", bufs=4, name=f"pj{bias_base}{hp}_{qc}")
                qsl = slice(qc * QC, (qc + 1) * QC)
                if QK_FP8 and w_tiles in (wq8_all, wk8_all):
                    for i in range(3):
                        nc.tensor.matmul(
                            ps[:],
                            w_tiles[:, i, :, hp * P : (hp + 1) * P],
                            x8_all[:, i, :, qsl],
                            start=(i == 0),
                            stop=(i == 2),
                            perf_mode=DR,
                        )
                else:
                    for kt in range(NXT):
                        nc.tensor.matmul(
                            ps[:],
                            w_tiles[kt][:, hp * P : (hp + 1) * P],
                            xt[kt][:, qsl],
                            start=(kt == 0),
                            stop=(kt == NXT - 1),
                        )
                nc.vector.tensor_scalar_add(
                    dst_tiles[hp][:, qsl], ps[:], pb_t[:, bias_base + hp : bias_base + hp + 1]
                )

            def vproj(c):
                """V[c] = (x_chunk @ Wv^T | ones) as [128, 6*65] bf16."""
                ps = psp.tile([P, QC], F32, tag="acc", bufs=4, name=f"vp{c}")
                for kt in range(NXT):
                    nc.tensor.matmul(
                        ps[:, 0:GD],
                        xt[kt][:, c * P : (c + 1) * P],
                        wv_t[kt][:],
                        start=(kt == 0),
                        stop=(kt == NXT - 1),
                    )
                vv = V[c].rearrange("p (h c) -> p h c", h=HLOC)
                nc.vector.tensor_copy(
                    vv[:, :, 0:DK], ps[:, 0:GD].rearrange("p (h c) -> p h c", h=HLOC)
                )
                nc.vector.memset(vv[:, :, DK : DK + 1], 1.0)

            def oproj(qc, mts):
                qsl = slice(qc * QC, (qc + 1) * QC)
                for mt in mts:
                    po = psp.tile([P, QC], F32, tag="acc", bufs=4, name=f"po{mt}_{qc}")
                    for j in range(NPT):
                        nc.tensor.matmul(
                            po[:],
                            wo_t[j][:, mt * P : (mt + 1) * P],
                            attn[j][:, qsl],
                            start=(j == 0),
                            stop=(j == NPT - 1),
                        )
                    osb = wp.tile([P, QC], BF16, tag="os", bufs=4, name=f"os{mt}_{qc}")
                    nc.vector.tensor_scalar_add(osb[:], po[:], pb_t[:, 6 + mt : 7 + mt])
                    nc.sync.dma_start(outT[mt * P : (mt + 1) * P, qsl], osb[:])

            # ---- emission helpers ----
            def oproj_mt(qc, mt):
                qsl = slice(qc * QC, (qc + 1) * QC)
                po = psp.tile([P, QC], F32, tag="acc", bufs=4, name=f"po{mt}_{qc}")
                for j in range(NPT):
                    nc.tensor.matmul(
                        po[:],
                        wo_t[j][:, mt * P : (mt + 1) * P],
                        attn[j][:, qsl],
                        start=(j == 0),
                        stop=(j == NPT - 1),
                    )
                osb = wp.tile([P, QC], BF16, tag="os", bufs=4, name=f"os{mt}_{qc}")
                nc.vector.tensor_scalar_add(osb[:], po[:], pb_t[:, 6 + mt : 7 + mt])
                nc.sync.dma_start(outT[mt * P : (mt + 1) * P, qsl], osb[:])

            def emit_pv(pv, c):
                E, atA, atB, hA, hB = pv["E"], pv["atA"], pv["atB"], pv["hA"], pv["hB"]
                nc.tensor.matmul(
                    atA[:], V[c][:, hA * VW : (hA + 1) * VW], E[c][:, 0:QC],
                    start=(c == 0), stop=(c == NKT - 1), skip_group_check=True,
                )
                nc.tensor.matmul(
                    atB[:], V[c][:, hB * VW : (hB + 1) * VW], E[c][:, QC : 2 * QC],
                    start=(c == 0), stop=(c == NKT - 1), skip_group_check=True,
                )

            def emit_normalize(pv):
                atA, atB = pv["atA"], pv["atB"]
                _qc, _hp = pv["qc"], pv["hp"]
                qsl = slice(_qc * QC, (_qc + 1) * QC)
                heads = []
                for at, rows in ((atA, slice(0, DK)), (atB, slice(DK, P))):
                    dsb = wp.tile([1, QC], F32, tag="dsb", bufs=4, name=f"d{_qc}_{_hp}_{rows.start}")
                    nc.vector.tensor_copy(dsb[:], at[DK:VW, :])
                    heads.append((at, rows, dsb))
                rs = []
                for at, rows, dsb in heads:
                    r = wp.tile([1, QC], F32, tag="r", bufs=4, name=f"r{_qc}_{_hp}_{rows.start}")
                    nc.vector.reciprocal_approx_fast(r[:], dsb[:])
                    rs.append(r)
                rbs = []
                for (at, rows, dsb), r in zip(heads, rs):
                    rb = wp.tile([DK, QC], F32, tag="rb", bufs=4, name=f"rb{_qc}_{_hp}_{rows.start}")
                    nc.gpsimd.partition_broadcast(rb[:], r[:], channels=DK)
                    rbs.append(rb)
                for (at, rows, dsb), rb in zip(heads, rbs):
                    nc.vector.tensor_mul(attn[_hp][rows, qsl], at[0:DK, :], rb[:])

            # ---- prefix: only what the first score chunks need ----
            # fused K(0,0)/Q(0,0): interleave MMs so both consume each x8
            # piece as it arrives instead of Q re-waiting the DMA chain
            if QK_FP8:
                psk = psp.tile([P, QC], F32, tag="acc", bufs=4, name="pfx_k")
                psq = psp.tile([P, QC], F32, tag="acc", bufs=4, name="pfx_q")
                for i in range(3):
                    nc.tensor.matmul(
                        psk[:], wk8_all[:, i, :, 0:P], x8_all[:, i, :, 0:QC],
                        start=(i == 0), stop=(i == 2), perf_mode=DR,
                    )
                    nc.tensor.matmul(
                        psq[:], wq8_all[:, i, :, 0:P], x8_all[:, i, :, 0:QC],
                        start=(i == 0), stop=(i == 2), perf_mode=DR,
                    )
                nc.vector.tensor_scalar_add(KT[0][:, 0:QC], psk[:], pb_t[:, 3:4])
                nc.vector.tensor_scalar_add(QT[0][:, 0:QC], psq[:], pb_t[:, 0:1])
            else:
                proj(KT, wk_t, 3, 0, 0)
                proj(QT, wq_t, 0, 0, 0)

            # ---- drip schedule: deferred PE work, one block per chunk slot ----
            drip = [[] for _ in range(12)]
            drip[0] = (
                [lambda q2=q2: proj(KT, wk8_all if QK_FP8 else wk_t, 3, 0, q2) for q2 in range(1, NQC)]
                + [lambda q2=q2: proj(KT, wk8_all if QK_FP8 else wk_t, 3, 1, q2) for q2 in range(NQC)]
                + [lambda: proj(QT, wq8_all if QK_FP8 else wq_t, 0, 1, 0)]
                + [lambda c=c: vproj(c) for c in range(8)]
            )
            drip[1] = (
                [lambda c=c: vproj(c) for c in range(8, 12)]
                + [lambda: proj(KT, wk8_all if QK_FP8 else wk_t, 3, 2, 0), lambda: proj(QT, wq8_all if QK_FP8 else wq_t, 0, 2, 0)]
                + [lambda c=c: vproj(c) for c in range(12, NKT)]
            )
            drip[2] = (
                [lambda q2=q2: proj(KT, wk8_all if QK_FP8 else wk_t, 3, 2, q2) for q2 in range(1, NQC)]
                + [lambda: proj(QT, wq8_all if QK_FP8 else wq_t, 0, 0, 1)]
            )
            drip[3] = [
                lambda: proj(QT, wq8_all if QK_FP8 else wq_t, 0, 1, 1),
                lambda: proj(QT, wq8_all if QK_FP8 else wq_t, 0, 2, 1),
                lambda: proj(QT, wq8_all if QK_FP8 else wq_t, 0, 0, 2),
            ]
            drip[4] = [lambda mt=mt: oproj_mt(0, mt) for mt in range(3)] + [
                lambda: proj(QT, wq8_all if QK_FP8 else wq_t, 0, 1, 2)
            ]
            drip[5] = [lambda mt=mt: oproj_mt(0, mt) for mt in range(3, NXT)] + [
                lambda: proj(QT, wq8_all if QK_FP8 else wq_t, 0, 2, 2), lambda: proj(QT, wq8_all if QK_FP8 else wq_t, 0, 0, 3)
            ]
            drip[6] = [lambda: proj(QT, wq8_all if QK_FP8 else wq_t, 0, 1, 3)]
            drip[7] = [lambda mt=mt: oproj_mt(1, mt) for mt in range(3)] + [
                lambda: proj(QT, wq8_all if QK_FP8 else wq_t, 0, 2, 3)
            ]
            drip[8] = [lambda mt=mt: oproj_mt(1, mt) for mt in range(3, NXT)]
            drip[10] = [lambda mt=mt: oproj_mt(2, mt) for mt in range(NXT)]

            # ---- attention: slot-pipelined emission ----
            prev = None
            iters = [(qc, hp) for qc in range(NQC) for hp in range(NPT)]
            for it_idx, (qc, hp) in enumerate(iters):
                qsl = slice(qc * QC, (qc + 1) * QC)
                hA, hB = 2 * hp, 2 * hp + 1
                cur = {
                    "E": [],
                    "atA": psp.tile([VW, QC], F32, tag="acc", bufs=4, name=f"atA{qc}_{hp}"),
                    "atB": psp.tile([VW, QC], F32, tag="acc", bufs=4, name=f"atB{qc}_{hp}"),
                    "hA": hA, "hB": hB, "qc": qc, "hp": hp,
                }
                dq = list(drip[it_idx])
                for c in range(NKT):
                    sAB = psp.tile([P, 2 * QC], F32, tag="sAB", bufs=2, name=f"s{qc}_{hp}_{c}")
                    ksl = slice(c * P, (c + 1) * P)
                    nc.tensor.matmul(sAB[:, 0:QC], KT[hp][0:DK, ksl], QT[hp][0:DK, qsl])
                    nc.tensor.matmul(
                        sAB[:, QC : 2 * QC], KT[hp][DK:P, ksl], QT[hp][DK:P, qsl]
                    )
                    e = ep.tile([P, 2 * QC], BF16, tag="E", bufs=20, name=f"e{qc}_{hp}_{c}")
                    nc.scalar.activation(e[:], sAB[:], EXP, scale=(1.0 / (DK * DK * 1.0)) if QK_FP8 else (1.0 / DK))
                    cur["E"].append(e)
                    if prev is not None:
                        emit_pv(prev, c)
                    if it_idx == len(iters) - 1 and c >= 1:
                        emit_pv(cur, c - 1)
                    if dq:
                        dq.pop(0)()
                while dq:
                    dq.pop(0)()
                if prev is not None:
                    emit_normalize(prev)
                prev = cur
            # flush last head-pair (chunks 0..14 already emitted in-loop)
            emit_pv(prev, NKT - 1)
            emit_normalize(prev)
            # keep PE warm through the normalize pause so oproj(3) runs at
            # full clock
            for i in range(10):
                nc.tensor.matmul(wps[:], ones_t[:], wrm[:], start=True, stop=True,
                                 skip_group_check=True)
            for mt in range(NXT):
                oproj_mt(3, mt)

    nc.compile()
    return nc


def make_in_maps(x, Wq, bq, Wk, bk, Wv, bv, Wo, bo):
    in_maps = []
    for c in range(NCORES):
        b, g = c // 2, c % 2
        sl = slice(g * GD, (g + 1) * GD)
        pbv = np.zeros((P, 12), np.float32)
        bo_eff = Wo[:, sl].astype(np.float64) @ bv[sl].astype(np.float64) + bo / 2.0
        qs = 8.0 if QK_FP8 else 1.0
        for j in range(NPT):
            pbv[:, 0 + j] = bq[sl][j * P : (j + 1) * P] * qs
            pbv[:, 3 + j] = bk[sl][j * P : (j + 1) * P] * qs
        for j in range(NXT):
            pbv[:, 6 + j] = bo_eff[j * P : (j + 1) * P]
        def blk(a, rows):
            # [rows*P, C] -> [P, rows*C] with block i at cols [i*C:(i+1)*C]
            r, cdim = a.shape
            return np.ascontiguousarray(
                a.reshape(rows, P, cdim).transpose(1, 0, 2).reshape(P, rows * cdim)
            )

        m = {}
        if QK_FP8:
            def dr_blk(a):
                # [768, C] -> [P, 3, 2, C] with d = i*256 + t*128 + p
                cdim = a.shape[1]
                return np.ascontiguousarray(
                    a.reshape(3, 2, P, cdim).transpose(2, 0, 1, 3).reshape(P, 3 * 2 * cdim)
                )
            m["x8T"] = dr_blk(x[b].T).astype(NPFP8)
            m["wq8T"] = dr_blk(Wq[sl, :].T * 8.0).astype(NPFP8)
            m["wk8T"] = dr_blk(Wk[sl, :].T * 8.0).astype(NPFP8)
        in_maps.append(
            {
                **m,
                "xT": blk(x[b].T, NXT).astype(NPBF16),
                "wqT": blk(Wq[sl, :].T, NXT).astype(NPBF16),
                "wkT": blk(Wk[sl, :].T, NXT).astype(NPBF16),
                "wvT": blk(Wv[sl, :].T, NXT).astype(NPBF16),
                "woT": blk(Wo[:, sl].T, NPT).astype(NPBF16),
                "pb": pbv,
            }
        )
    return in_maps


def kernel(x, Wq, bq, Wk, bk, Wv, bv, Wo, bo, _trace=False):
    x = np.asarray(x, np.float32)
    args = [np.asarray(a, np.float32) for a in (Wq, bq, Wk, bk, Wv, bv, Wo, bo)]
    if "nc" not in _NC_CACHE:
        _NC_CACHE["nc"] = build_nc()
    nc = _NC_CACHE["nc"]
    in_maps = make_in_maps(x, *args)
    res = run_bass_kernel_spmd(nc, in_maps, core_ids=list(range(NCORES)), trace=_trace)
    _NC_CACHE["last_result"] = res
    out = np.empty((B, S, DM), np.float32)
    for b in range(B):
        out[b] = (
            res.results[2 * b]["outT"].astype(np.float32)
            + res.results[2 * b + 1]["outT"].astype(np.float32)
        ).T
    return out
